# revision 1
# baseline (speedup 1.0000x reference)
"""Trainium2 Bass kernel for nn_CalibratedNormFixedAlpha (moe_routing).

Math (reference):
  out = (1-a)*x_global + a*x_groups,  a = 0.5
  x_global = (x - mu_g[c]) * (gamma_global[c] * rsqrt(var_g[c]+eps)) + beta_global[c]
             with mu_g/var_g per-channel over (N,H,W)  [biased var]
  x_groups = (x - mu_s[b,c]) * rsqrt(var_s[b,c]+eps) * g[b,c] + bt[b,c]
             with mu_s/var_s per-(sample,channel) over (H,W)
             g/bt routed from [G,C] tables by labels[b] % G

Everything is affine in x per (b,c):
  out = A[b,c] * x + B[b,c]
  A = 0.5*sg[c] + 0.5*rs[b,c]*g[b,c]
  B = 0.5*(beta_global[c] - mu_g[c]*sg[c]) + 0.5*(bt[b,c] - mu_s[b,c]*rs[b,c]*g[b,c])
  sg = gamma_global*rsqrt(var_g+eps), rs = rsqrt(var_s+eps)

Sharding: data-parallel over batch, 8 samples/core. Per-core shard
(8*64=512 (b,c) planes x 12544 spatial) kept fully resident in SBUF as
4 tiles of [128, 12544] f32 (2 samples x 64 ch per tile -> 128 partitions).
Pass 1: bn_stats/bn_aggr per partition -> (mean, E[x^2]) per (b,c).
Tiny [4,128,2] AllReduce over the 8 cores gives global per-channel stats.
Pass 2: out = A*x + B per partition (DVE tensor_scalar / ACT activation),
stored straight from SBUF. HBM traffic = 1 read + 1 write of x.
"""

import numpy as np

# -------- problem constants (hardcoded per contract) --------
B, C, H, W = 64, 64, 112, 112
HW = H * W                 # 12544
N_CORES = 8
B_LOC = B // N_CORES       # 8 samples per core
P = 128                    # SBUF partitions
NT = (B_LOC * C) // P      # 4 tiles of [128, HW] per core
CHUNK = 1792               # DMA / affine chunk along free dim (7 per tile)
NCH = HW // CHUNK          # 7
SUB = 448                  # bn_stats subgroup (<=512, uniform size)
NSUB = CHUNK // SUB        # 4
EPS = 1e-5
ALPHA = 0.5
NUM_GROUPS = 32

_STATE = {}


def _build_module(chunk=CHUNK, sub=SUB, act_mod=0, use_collective=True,
                  load_eng="sync", store_eng="sync", store_chunk=None, reps=1,
                  split_cc=False, use_allgather=True, cc_dma_eng="sync",
                  first_tile_act=True):
    """act_mod: affine chunk idx %% act_mod == act_mod-1 goes to ACT (0 = all DVE)."""
    import concourse.bass as bass
    import concourse.bacc as bacc
    import concourse.tile as tile
    from concourse import mybir

    nch = HW // chunk
    nsub = chunk // sub
    if store_chunk is None:
        store_chunk = chunk
    f32 = mybir.dt.float32
    nc = bacc.Bacc(
        "TRN2",
        target_bir_lowering=False,
        debug=False,
        num_devices=N_CORES,
        dynamic_dma_scratch_size=8192,
    )

    x_h = nc.dram_tensor("x", [B_LOC * C, HW], f32, kind="ExternalInput")
    gg_h = nc.dram_tensor("gg", [P, NT], f32, kind="ExternalInput")     # routed gamma per (b,c)
    bg_h = nc.dram_tensor("bg", [P, NT], f32, kind="ExternalInput")     # routed beta per (b,c)
    ggl_h = nc.dram_tensor("gglob", [P, 1], f32, kind="ExternalInput")  # gamma_global tiled x2
    bgl_h = nc.dram_tensor("bglob", [P, 1], f32, kind="ExternalInput")  # beta_global tiled x2
    out_h = nc.dram_tensor("out", [B_LOC * C, HW], f32, kind="ExternalOutput")

    x_ap = x_h.ap()
    out_ap = out_h.ap()
    Sqrt = mybir.ActivationFunctionType.Sqrt
    Identity = mybir.ActivationFunctionType.Identity
    add = mybir.AluOpType.add
    mult = mybir.AluOpType.mult

    with tile.TileContext(nc) as tc:
        with (
            tc.tile_pool(name="xp", bufs=1) as xp,
            tc.tile_pool(name="sp", bufs=1) as sp,
            tc.tile_pool(name="dp", bufs=1, space="DRAM") as dp,
        ):
            def emit_body():
                # small replicated inputs
                gg_sb = sp.tile([P, NT], f32, tag="gg")
                bg_sb = sp.tile([P, NT], f32, tag="bg")
                ggl_sb = sp.tile([P, 1], f32, tag="ggl")
                bgl_sb = sp.tile([P, 1], f32, tag="bgl")
                nc.gpsimd.dma_start(out=gg_sb[:, :], in_=gg_h.ap())
                nc.gpsimd.dma_start(out=bg_sb[:, :], in_=bg_h.ap())
                nc.gpsimd.dma_start(out=ggl_sb[:, :], in_=ggl_h.ap())
                nc.gpsimd.dma_start(out=bgl_sb[:, :], in_=bgl_h.ap())
                eps_sb = sp.tile([P, 1], f32, tag="eps", name="eps")
                nc.vector.memset(eps_sb[:, :], EPS)

                # collective bounce buffers (DRAM, tiny)
                if use_allgather:
                    cc_in = dp.tile([P, 2], f32, tag="ccin")
                    cc_out = dp.tile([N_CORES, P, 2], f32, tag="ccout")
                else:
                    cc_in = dp.tile([NT, P, 2], f32, tag="ccin")
                    cc_out = dp.tile([NT, P, 2], f32, tag="ccout")

                xt = [xp.tile([P, HW], f32, tag=f"x{t}", name=f"x{t}") for t in range(NT)]
                stats = [sp.tile([P, nch * nsub, 6], f32, tag=f"st{t}", name=f"st{t}") for t in range(NT)]
                mv = [sp.tile([P, 2], f32, tag=f"mv{t}", name=f"mv{t}") for t in range(NT)]
                pre = [sp.tile([P, 1], f32, tag=f"pre{t}", name=f"pre{t}") for t in range(NT)]  # 0.5*rs*g
                c2 = [sp.tile([P, 1], f32, tag=f"c2{t}", name=f"c2{t}") for t in range(NT)]    # 0.5*(bt - mean*rs*g)

                # ---- pass 1: load + per-(b,c) stats ----
                pks = [None] * NT
                for t in range(NT):
                    rows = slice(t * P, (t + 1) * P)
                    for ch in range(nch):
                        sl = slice(ch * chunk, (ch + 1) * chunk)
                        le = (["sync", "scalar"][(t * nch + ch) % 2]
                              if load_eng == "alt" else load_eng)
                        last = t == NT - 1 and ch == nch - 1
                        if last:
                            # final chunk arrives as per-subgroup mini-loads so the
                            # last bn_stats drains right behind the last bytes
                            for s in range(nsub):
                                ssl = slice(ch * chunk + s * sub, ch * chunk + (s + 1) * sub)
                                getattr(nc, le).dma_start(
                                    out=xt[t][:, ssl], in_=x_ap[rows, ssl]
                                )
                                nc.vector.bn_stats(
                                    out=stats[t][:, ch * nsub + s, :], in_=xt[t][:, ssl]
                                )
                        else:
                            getattr(nc, le).dma_start(out=xt[t][:, sl], in_=x_ap[rows, sl])
                            for s in range(nsub):
                                ssl = slice(ch * chunk + s * sub, ch * chunk + (s + 1) * sub)
                                nc.vector.bn_stats(
                                    out=stats[t][:, ch * nsub + s, :], in_=xt[t][:, ssl]
                                )
                    nc.vector.bn_aggr(out=mv[t][:, :], in_=stats[t][:, :, :])

                    # pack (mean, E[x^2]) for the all-reduce
                    msq = sp.tile([P, 1], f32, tag=f"msq{t}", name=f"msq{t}")
                    nc.vector.tensor_mul(out=msq[:, :], in0=mv[t][:, 0:1], in1=mv[t][:, 0:1])
                    pk = sp.tile([P, 2], f32, tag=f"pk{t}", name=f"pk{t}")
                    nc.vector.tensor_copy(out=pk[:, 0:1], in_=mv[t][:, 0:1])
                    nc.vector.tensor_add(out=pk[:, 1:2], in0=mv[t][:, 1:2], in1=msq[:, :])
                    if use_allgather:
                        # running local sum over tiles; only the final tiny add
                        # plus one 1KB DMA sits after the last tile's stats
                        if t == 0:
                            pks[0] = pk
                        else:
                            acc = sp.tile([P, 2], f32, tag=f"pka{t}", name=f"pka{t}")
                            nc.vector.tensor_add(
                                out=acc[:, :], in0=pks[t - 1][:, :], in1=pk[:, :]
                            )
                            pks[t] = acc
                        if t == NT - 1:
                            getattr(nc, cc_dma_eng).dma_start(
                                out=cc_in[:, :], in_=pks[t][:, :]
                            )
                    else:
                        nc.gpsimd.dma_start(out=cc_in[t, :, :], in_=pk[:, :])

                    # local coefficient pieces (independent of global stats)
                    sd = sp.tile([P, 1], f32, tag=f"sd{t}", name=f"sd{t}")
                    nc.scalar.activation(out=sd[:, :], in_=mv[t][:, 1:2], func=Sqrt, bias=eps_sb[:, 0:1])
                    rs = sp.tile([P, 1], f32, tag=f"rs{t}", name=f"rs{t}")
                    nc.vector.reciprocal(out=rs[:, :], in_=sd[:, :])
                    t1 = sp.tile([P, 1], f32, tag=f"t1_{t}", name=f"t1_{t}")
                    nc.vector.tensor_mul(out=t1[:, :], in0=rs[:, :], in1=gg_sb[:, t : t + 1])
                    nc.vector.tensor_scalar_mul(out=pre[t][:, :], in0=t1[:, :], scalar1=ALPHA)
                    mB = sp.tile([P, 1], f32, tag=f"mB{t}", name=f"mB{t}")
                    nc.vector.tensor_mul(out=mB[:, :], in0=mv[t][:, 0:1], in1=t1[:, :])
                    c2a = sp.tile([P, 1], f32, tag=f"c2a{t}", name=f"c2a{t}")
                    nc.vector.tensor_sub(out=c2a[:, :], in0=bg_sb[:, t : t + 1], in1=mB[:, :])
                    nc.vector.tensor_scalar_mul(out=c2[t][:, :], in0=c2a[:, :], scalar1=ALPHA)

                # ---- tiny cross-core exchange of (sum-able) stats ----
                if use_collective and use_allgather:
                    # AllGather the raw per-core packs (1KB/core) and sum the
                    # 8 core-slices locally: roughly half the latency of the
                    # ncfw AllReduce for latency-bound payloads.
                    nc.gpsimd.collective_compute(
                        "AllGather",
                        mybir.AluOpType.bypass,
                        replica_groups=[list(range(N_CORES))],
                        ins=[cc_in.opt()],
                        outs=[cc_out.opt()],
                    )
                elif use_collective and split_cc:
                    # two pipelined all-reduces: tiles 0..NT-2 can launch while
                    # the last tile is still loading; only the second (1-tile)
                    # collective sits on the critical path
                    nc.gpsimd.collective_compute(
                        "AllReduce",
                        add,
                        replica_groups=[list(range(N_CORES))],
                        ins=[cc_in[0 : NT - 1, :, :].opt()],
                        outs=[cc_out[0 : NT - 1, :, :].opt()],
                    )
                    nc.gpsimd.collective_compute(
                        "AllReduce",
                        add,
                        replica_groups=[list(range(N_CORES))],
                        ins=[cc_in[NT - 1 : NT, :, :].opt()],
                        outs=[cc_out[NT - 1 : NT, :, :].opt()],
                    )
                elif use_collective:
                    nc.gpsimd.collective_compute(
                        "AllReduce",
                        add,
                        replica_groups=[list(range(N_CORES))],
                        ins=[cc_in.opt()],
                        outs=[cc_out.opt()],
                    )
                else:  # timing-only variant: skip the collective, copy in->out
                    nc.gpsimd.dma_start(out=cc_out[:, :, :], in_=cc_in[:, :, :])

                # gstats: all (core, tile, s) stat pairs per channel, broadcast
                # over the partition halves (p//64)
                if use_allgather:
                    gstats = sp.tile([P, 2, N_CORES, 2], f32, tag="gstats")
                    gsrc = cc_out[:, :, :].rearrange("r (s c) st -> c st r s", s=2)
                    for h in range(2):
                        getattr(nc, cc_dma_eng).dma_start(
                            out=gstats[h * 64 : (h + 1) * 64, :, :, :], in_=gsrc
                        )
                else:
                    gstats = sp.tile([P, 2, NT, 2], f32, tag="gstats")
                    gsrc = cc_out[:, :, :].rearrange("t (s c) st -> c st t s", s=2)
                    for h in range(2):
                        nc.gpsimd.dma_start(
                            out=gstats[h * 64 : (h + 1) * 64, :, :, :], in_=gsrc
                        )

                # global per-channel coefficients
                def stile(tag):
                    return sp.tile([P, 1], f32, tag=tag, name=tag)

                Sm, Sm2 = stile("Sm"), stile("Sm2")
                if use_allgather:
                    nc.vector.tensor_reduce(
                        out=Sm[:, :], in_=gstats[:, 0, :, :],
                        axis=mybir.AxisListType.XY, op=add,
                    )
                    nc.vector.tensor_reduce(
                        out=Sm2[:, :], in_=gstats[:, 1, :, :],
                        axis=mybir.AxisListType.XY, op=add,
                    )
                else:
                    nc.vector.tensor_reduce(
                        out=Sm[:, :], in_=gstats[:, 0, :, :], axis=mybir.AxisListType.XY, op=add
                    )
                    nc.vector.tensor_reduce(
                        out=Sm2[:, :], in_=gstats[:, 1, :, :], axis=mybir.AxisListType.XY, op=add
                    )
                mu_g, m2g = stile("mu_g"), stile("m2g")
                nc.vector.tensor_scalar_mul(out=mu_g[:, :], in0=Sm[:, :], scalar1=1.0 / B)
                nc.vector.tensor_scalar_mul(out=m2g[:, :], in0=Sm2[:, :], scalar1=1.0 / B)
                musq, varg = stile("musq"), stile("varg")
                nc.vector.tensor_mul(out=musq[:, :], in0=mu_g[:, :], in1=mu_g[:, :])
                nc.vector.tensor_sub(out=varg[:, :], in0=m2g[:, :], in1=musq[:, :])
                sdg, invg = stile("sdg"), stile("invg")
                nc.scalar.activation(out=sdg[:, :], in_=varg[:, :], func=Sqrt, bias=eps_sb[:, 0:1])
                nc.vector.reciprocal(out=invg[:, :], in_=sdg[:, :])
                sg, sgh = stile("sg"), stile("sgh")
                nc.vector.tensor_mul(out=sg[:, :], in0=ggl_sb[:, :], in1=invg[:, :])
                nc.vector.tensor_scalar_mul(out=sgh[:, :], in0=sg[:, :], scalar1=1.0 - ALPHA)
                Kt, K, Kh = stile("Kt"), stile("K"), stile("Kh")
                nc.vector.tensor_mul(out=Kt[:, :], in0=mu_g[:, :], in1=sg[:, :])
                nc.vector.tensor_sub(out=K[:, :], in0=bgl_sb[:, :], in1=Kt[:, :])
                nc.vector.tensor_scalar_mul(out=Kh[:, :], in0=K[:, :], scalar1=1.0 - ALPHA)

                A = [stile(f"A{t}") for t in range(NT)]
                Bc = [stile(f"B{t}") for t in range(NT)]
                for t in range(NT):
                    nc.vector.tensor_add(out=A[t][:, :], in0=sgh[:, :], in1=pre[t][:, :])
                    nc.vector.tensor_add(out=Bc[t][:, :], in0=Kh[:, :], in1=c2[t][:, :])

                # ---- pass 2: out = A*x + B, store ----
                for t in range(NT):
                    rows = slice(t * P, (t + 1) * P)
                    for ch in range(nch):
                        sl = slice(ch * chunk, (ch + 1) * chunk)
                        idx = t * nch + ch
                        on_act = (act_mod and idx % act_mod == act_mod - 1) or (
                            first_tile_act and t == 0
                        )
                        if on_act:
                            nc.scalar.activation(
                                out=xt[t][:, sl],
                                in_=xt[t][:, sl],
                                func=Identity,
                                bias=Bc[t][:, 0:1],
                                scale=A[t][:, 0:1],
                            )
                        else:
                            nc.vector.tensor_scalar(
                                out=xt[t][:, sl],
                                in0=xt[t][:, sl],
                                scalar1=A[t][:, 0:1],
                                scalar2=Bc[t][:, 0:1],
                                op0=mult,
                                op1=add,
                            )
                    for sc in range(HW // store_chunk):
                        ssl = slice(sc * store_chunk, (sc + 1) * store_chunk)
                        se = (["scalar", "sync"][(t * (HW // store_chunk) + sc) % 2]
                              if store_eng == "alt" else store_eng)
                        getattr(nc, se).dma_start(
                            out=out_ap[rows, ssl], in_=xt[t][:, ssl]
                        )

            for _rep in range(reps):
                emit_body()

    nc.compile()
    return nc


def _get_nc():
    if "nc" not in _STATE:
        _STATE["nc"] = _build_module()
    return _STATE["nc"]


def kernel(x, gamma_global, beta_global, gamma_groups, beta_groups, labels,
           _trace=False):
    from concourse.bass_utils import run_bass_kernel_spmd

    nc = _get_nc()

    x = np.asarray(x, dtype=np.float32)
    gamma_global = np.asarray(gamma_global, dtype=np.float32)
    beta_global = np.asarray(beta_global, dtype=np.float32)
    gamma_groups = np.asarray(gamma_groups, dtype=np.float32)
    beta_groups = np.asarray(beta_groups, dtype=np.float32)
    labels = np.asarray(labels)

    # host-side routing (tiny): per-(b,c) affine params
    gidx = (labels.astype(np.int64) % NUM_GROUPS)
    ggf = gamma_groups[gidx]  # [B, C]
    bgf = beta_groups[gidx]   # [B, C]

    ggl2 = np.ascontiguousarray(np.tile(gamma_global, 2).reshape(P, 1))
    bgl2 = np.ascontiguousarray(np.tile(beta_global, 2).reshape(P, 1))

    in_maps = []
    for i in range(N_CORES):
        rows = slice(i * B_LOC, (i + 1) * B_LOC)
        in_maps.append(
            {
                "x": np.ascontiguousarray(x[rows].reshape(B_LOC * C, HW)),
                "gg": np.ascontiguousarray(ggf[rows].reshape(NT, P).T),
                "bg": np.ascontiguousarray(bgf[rows].reshape(NT, P).T),
                "gglob": ggl2,
                "bglob": bgl2,
            }
        )

    res = run_bass_kernel_spmd(
        nc, in_maps, core_ids=list(range(N_CORES)), trace=_trace
    )
    _STATE["last_res"] = res

    out = np.empty((B, C, H, W), dtype=np.float32)
    for i in range(N_CORES):
        out[i * B_LOC : (i + 1) * B_LOC] = res.results[i]["out"].reshape(
            B_LOC, C, H, W
        )
    return out



# revision 3
# speedup vs baseline: 1.1456x; 1.1456x over previous
"""Trainium2 Bass kernel for nn_CalibratedNormFixedAlpha (moe_routing).

Math (reference):
  out = (1-a)*x_global + a*x_groups,  a = 0.5
  x_global = (x - mu_g[c]) * (gamma_global[c] * rsqrt(var_g[c]+eps)) + beta_global[c]
             with mu_g/var_g per-channel over (N,H,W)  [biased var]
  x_groups = (x - mu_s[b,c]) * rsqrt(var_s[b,c]+eps) * g[b,c] + bt[b,c]
             with mu_s/var_s per-(sample,channel) over (H,W)
             g/bt routed from [G,C] tables by labels[b] % G

Everything is affine in x per (b,c):
  out = A[b,c] * x + B[b,c]
  A = 0.5*sg[c] + 0.5*rs[b,c]*g[b,c]
  B = 0.5*(beta_global[c] - mu_g[c]*sg[c]) + 0.5*(bt[b,c] - mu_s[b,c]*rs[b,c]*g[b,c])
  sg = gamma_global*rsqrt(var_g+eps), rs = rsqrt(var_s+eps)

Sharding: CHANNEL-parallel (8 channels/core, all 64 samples). Both stat
families are then core-local: per-(b,c) spatial stats live on one core, and
the global per-channel (N,H,W) stats only mix the 64 sample-planes of a
channel that all sit on the same core -> ZERO collectives.

Per-core shard: 512 (b,c) planes x 12544 spatial, plane = b*8 + c_local,
resident in SBUF as 4 tiles of [128, 12544] f32. Pass 1: bn_stats/bn_aggr
per partition -> (mean, var) per plane; packsum accumulates (mean, E[x^2])
over tiles. The cross-plane (partition-dim) reduction per channel is one
tiny PE matmul with a 0/1 mask (mask8[p,c] = [p%8==c]); a second tiny
matmul broadcasts the per-channel global coefficients back to all 128
partitions. Pass 2: out = A*x + B per partition, stored from SBUF.
HBM traffic = 1 read + 1 write of x.
"""

import numpy as np

# -------- problem constants (hardcoded per contract) --------
B, C, H, W = 64, 64, 112, 112
HW = H * W                 # 12544
N_CORES = 8
C_LOC = C // N_CORES       # 8 channels per core
P = 128                    # SBUF partitions
NPL = B * C_LOC            # 512 planes per core
NT = NPL // P              # 4 tiles of [128, HW]
CHUNK = 1792               # DMA / affine chunk along free dim (7 per tile)
NCH = HW // CHUNK          # 7
SUB = 448                  # bn_stats subgroup (<=512, uniform size)
NSUB = CHUNK // SUB        # 4
EPS = 1e-5
ALPHA = 0.5
NUM_GROUPS = 32

_STATE = {}


def _build_module(chunk=CHUNK, sub=SUB, load_eng="sync", store_eng="sync",
                  store_chunk=None, first_tile_act=True, first_store_split=4):
    import concourse.bass as bass
    import concourse.bacc as bacc
    import concourse.tile as tile
    from concourse import mybir

    nch = HW // chunk
    nsub = chunk // sub
    if store_chunk is None:
        store_chunk = chunk
    f32 = mybir.dt.float32
    nc = bacc.Bacc(
        "TRN2",
        target_bir_lowering=False,
        debug=False,
        num_devices=N_CORES,
        dynamic_dma_scratch_size=8192,
    )

    x_h = nc.dram_tensor("x", [NPL, HW], f32, kind="ExternalInput")
    gg_h = nc.dram_tensor("gg", [P, NT], f32, kind="ExternalInput")     # routed gamma per plane
    bg_h = nc.dram_tensor("bg", [P, NT], f32, kind="ExternalInput")     # routed beta per plane
    ggl_h = nc.dram_tensor("gglh", [8, 1], f32, kind="ExternalInput")   # 0.5*gamma_global[core ch]
    bgl_h = nc.dram_tensor("bglh", [8, 1], f32, kind="ExternalInput")   # 0.5*beta_global[core ch]
    m8_h = nc.dram_tensor("mask8", [P, 8], f32, kind="ExternalInput")   # [p%8==c]
    m8t_h = nc.dram_tensor("mask8t", [8, P], f32, kind="ExternalInput")  # transpose
    out_h = nc.dram_tensor("out", [NPL, HW], f32, kind="ExternalOutput")

    x_ap = x_h.ap()
    out_ap = out_h.ap()
    Sqrt = mybir.ActivationFunctionType.Sqrt
    Identity = mybir.ActivationFunctionType.Identity
    add = mybir.AluOpType.add
    mult = mybir.AluOpType.mult

    with tile.TileContext(nc) as tc:
        with (
            tc.tile_pool(name="xp", bufs=1) as xp,
            tc.tile_pool(name="sp", bufs=1) as sp,
            tc.tile_pool(name="pp", bufs=1, space="PSUM") as pp,
        ):
            # small replicated inputs
            gg_sb = sp.tile([P, NT], f32, tag="gg")
            bg_sb = sp.tile([P, NT], f32, tag="bg")
            ggl_sb = sp.tile([8, 1], f32, tag="ggl")
            bgl_sb = sp.tile([8, 1], f32, tag="bgl")
            m8_sb = sp.tile([P, 8], f32, tag="m8")
            m8t_sb = sp.tile([8, P], f32, tag="m8t")
            nc.gpsimd.dma_start(out=gg_sb[:, :], in_=gg_h.ap())
            nc.gpsimd.dma_start(out=bg_sb[:, :], in_=bg_h.ap())
            nc.gpsimd.dma_start(out=ggl_sb[:, :], in_=ggl_h.ap())
            nc.gpsimd.dma_start(out=bgl_sb[:, :], in_=bgl_h.ap())
            nc.gpsimd.dma_start(out=m8_sb[:, :], in_=m8_h.ap())
            nc.gpsimd.dma_start(out=m8t_sb[:, :], in_=m8t_h.ap())
            eps_sb = sp.tile([P, 1], f32, tag="eps", name="eps")
            nc.vector.memset(eps_sb[:, :], EPS)

            xt = [xp.tile([P, HW], f32, tag=f"x{t}", name=f"x{t}") for t in range(NT)]
            stats = [sp.tile([P, nch * nsub, 6], f32, tag=f"st{t}", name=f"st{t}")
                     for t in range(NT)]
            mv = [sp.tile([P, 2], f32, tag=f"mv{t}", name=f"mv{t}") for t in range(NT)]
            pre_all = sp.tile([P, NT], f32, tag="pre")   # 0.5*rs*g per tile col
            c2_all = sp.tile([P, NT], f32, tag="c2")     # 0.5*(bt - mean*rs*g)
            pks = [None] * NT

            # ---- pass 1: load + per-plane stats ----
            for t in range(NT):
                rows = slice(t * P, (t + 1) * P)
                for ch in range(nch):
                    sl = slice(ch * chunk, (ch + 1) * chunk)
                    last = t == NT - 1 and ch == nch - 1
                    if last:
                        # final chunk arrives as per-subgroup mini-loads so the
                        # last bn_stats drains right behind the last bytes
                        for s in range(nsub):
                            ssl = slice(ch * chunk + s * sub, ch * chunk + (s + 1) * sub)
                            getattr(nc, load_eng).dma_start(
                                out=xt[t][:, ssl], in_=x_ap[rows, ssl]
                            )
                            nc.vector.bn_stats(
                                out=stats[t][:, ch * nsub + s, :], in_=xt[t][:, ssl]
                            )
                    else:
                        getattr(nc, load_eng).dma_start(out=xt[t][:, sl], in_=x_ap[rows, sl])
                        for s in range(nsub):
                            ssl = slice(ch * chunk + s * sub, ch * chunk + (s + 1) * sub)
                            nc.vector.bn_stats(
                                out=stats[t][:, ch * nsub + s, :], in_=xt[t][:, ssl]
                            )
                nc.vector.bn_aggr(out=mv[t][:, :], in_=stats[t][:, :, :])

                # pack (mean, E[x^2]) and accumulate over tiles
                msq = sp.tile([P, 1], f32, tag=f"msq{t}", name=f"msq{t}")
                nc.vector.tensor_mul(out=msq[:, :], in0=mv[t][:, 0:1], in1=mv[t][:, 0:1])
                pk = sp.tile([P, 2], f32, tag=f"pk{t}", name=f"pk{t}")
                nc.vector.tensor_copy(out=pk[:, 0:1], in_=mv[t][:, 0:1])
                nc.vector.tensor_add(out=pk[:, 1:2], in0=mv[t][:, 1:2], in1=msq[:, :])
                if t == 0:
                    pks[0] = pk
                else:
                    acc = sp.tile([P, 2], f32, tag=f"pka{t}", name=f"pka{t}")
                    nc.vector.tensor_add(out=acc[:, :], in0=pks[t - 1][:, :], in1=pk[:, :])
                    pks[t] = acc

                # local coefficient pieces (independent of global stats)
                sd = sp.tile([P, 1], f32, tag=f"sd{t}", name=f"sd{t}")
                nc.scalar.activation(out=sd[:, :], in_=mv[t][:, 1:2], func=Sqrt, bias=eps_sb[:, 0:1])
                rs = sp.tile([P, 1], f32, tag=f"rs{t}", name=f"rs{t}")
                nc.vector.reciprocal(out=rs[:, :], in_=sd[:, :])
                t1 = sp.tile([P, 1], f32, tag=f"t1_{t}", name=f"t1_{t}")
                nc.vector.tensor_mul(out=t1[:, :], in0=rs[:, :], in1=gg_sb[:, t : t + 1])
                nc.vector.tensor_scalar_mul(out=pre_all[:, t : t + 1], in0=t1[:, :], scalar1=ALPHA)
                mB = sp.tile([P, 1], f32, tag=f"mB{t}", name=f"mB{t}")
                nc.vector.tensor_mul(out=mB[:, :], in0=mv[t][:, 0:1], in1=t1[:, :])
                c2a = sp.tile([P, 1], f32, tag=f"c2a{t}", name=f"c2a{t}")
                nc.vector.tensor_sub(out=c2a[:, :], in0=bg_sb[:, t : t + 1], in1=mB[:, :])
                nc.vector.tensor_scalar_mul(out=c2_all[:, t : t + 1], in0=c2a[:, :], scalar1=ALPHA)

            # ---- per-channel global stats via mask matmul (PE) ----
            s8_ps = pp.tile([8, 2], f32, tag="s8")
            nc.tensor.matmul(s8_ps[:, :], m8_sb[:, :], pks[NT - 1][:, :])

            # chain on 8 partitions: mu_g=S/64, m2=S2/64, varg=m2-mu^2,
            # sgh=0.5*gg*rsqrt(varg+eps), Kh=0.5*bg - mu*sgh
            mm = sp.tile([8, 2], f32, tag="mm")
            nc.vector.tensor_scalar_mul(out=mm[:, :], in0=s8_ps[:, :], scalar1=1.0 / B)
            musq = sp.tile([8, 1], f32, tag="musq")
            nc.vector.tensor_mul(out=musq[:, :], in0=mm[:, 0:1], in1=mm[:, 0:1])
            varg = sp.tile([8, 1], f32, tag="varg")
            nc.vector.tensor_sub(out=varg[:, :], in0=mm[:, 1:2], in1=musq[:, :])
            sdg = sp.tile([8, 1], f32, tag="sdg")
            nc.scalar.activation(out=sdg[:, :], in_=varg[:, :], func=Sqrt, bias=eps_sb[0:8, 0:1])
            invg = sp.tile([8, 1], f32, tag="invg")
            nc.vector.reciprocal(out=invg[:, :], in_=sdg[:, :])
            gk = sp.tile([8, 2], f32, tag="gk")
            nc.vector.tensor_mul(out=gk[:, 0:1], in0=ggl_sb[:, :], in1=invg[:, :])  # sgh
            kt = sp.tile([8, 1], f32, tag="kt")
            nc.vector.tensor_mul(out=kt[:, :], in0=mm[:, 0:1], in1=gk[:, 0:1])
            nc.vector.tensor_sub(out=gk[:, 1:2], in0=bgl_sb[:, :], in1=kt[:, :])    # Kh

            # broadcast per-channel (sgh, Kh) to all 128 partitions (PE)
            gkb_ps = pp.tile([P, 2], f32, tag="gkb")
            nc.tensor.matmul(gkb_ps[:, :], m8t_sb[:, :], gk[:, :])
            gkb = sp.tile([P, 2], f32, tag="gkbs")
            nc.scalar.activation(out=gkb[:, :], in_=gkb_ps[:, :], func=Identity)

            # final per-(plane,tile) affine coefficients
            A_all = sp.tile([P, NT], f32, tag="Aall")
            B_all = sp.tile([P, NT], f32, tag="Ball")
            nc.vector.tensor_scalar_add(out=A_all[:, :], in0=pre_all[:, :], scalar1=gkb[:, 0:1])
            nc.vector.tensor_scalar_add(out=B_all[:, :], in0=c2_all[:, :], scalar1=gkb[:, 1:2])

            # ---- pass 2: out = A*x + B, store ----
            for t in range(NT):
                rows = slice(t * P, (t + 1) * P)
                for ch in range(nch):
                    sl = slice(ch * chunk, (ch + 1) * chunk)
                    first = t == 0 and ch == 0
                    on_act = first_tile_act and t == 0
                    if first and first_store_split > 1:
                        # split the first chunk so the store pipeline launches
                        # as early as possible
                        fs = chunk // first_store_split
                        for s in range(first_store_split):
                            fsl = slice(s * fs, (s + 1) * fs)
                            nc.scalar.activation(
                                out=xt[t][:, fsl], in_=xt[t][:, fsl], func=Identity,
                                bias=B_all[:, t : t + 1], scale=A_all[:, t : t + 1],
                            )
                            getattr(nc, store_eng).dma_start(
                                out=out_ap[rows, fsl], in_=xt[t][:, fsl]
                            )
                        continue
                    if on_act:
                        nc.scalar.activation(
                            out=xt[t][:, sl], in_=xt[t][:, sl], func=Identity,
                            bias=B_all[:, t : t + 1], scale=A_all[:, t : t + 1],
                        )
                    else:
                        nc.vector.tensor_scalar(
                            out=xt[t][:, sl], in0=xt[t][:, sl],
                            scalar1=A_all[:, t : t + 1], scalar2=B_all[:, t : t + 1],
                            op0=mult, op1=add,
                        )
                    for sc in range(chunk // store_chunk):
                        ssl = slice(ch * chunk + sc * store_chunk,
                                    ch * chunk + (sc + 1) * store_chunk)
                        getattr(nc, store_eng).dma_start(
                            out=out_ap[rows, ssl], in_=xt[t][:, ssl]
                        )

    nc.compile()
    return nc


def _get_nc():
    if "nc" not in _STATE:
        _STATE["nc"] = _build_module()
    return _STATE["nc"]


def kernel(x, gamma_global, beta_global, gamma_groups, beta_groups, labels,
           _trace=False):
    from concourse.bass_utils import run_bass_kernel_spmd

    nc = _get_nc()

    x = np.asarray(x, dtype=np.float32)
    gamma_global = np.asarray(gamma_global, dtype=np.float32)
    beta_global = np.asarray(beta_global, dtype=np.float32)
    gamma_groups = np.asarray(gamma_groups, dtype=np.float32)
    beta_groups = np.asarray(beta_groups, dtype=np.float32)
    labels = np.asarray(labels)

    # host-side routing (tiny): per-(b,c) affine params
    gidx = (labels.astype(np.int64) % NUM_GROUPS)
    ggf = gamma_groups[gidx]  # [B, C]
    bgf = beta_groups[gidx]   # [B, C]

    pidx = np.arange(P)
    m8 = (pidx[:, None] % 8 == np.arange(8)[None, :]).astype(np.float32)
    m8t = np.ascontiguousarray(m8.T)

    in_maps = []
    for i in range(N_CORES):
        cols = slice(i * C_LOC, (i + 1) * C_LOC)
        # plane = b*8 + c_local
        in_maps.append(
            {
                "x": np.ascontiguousarray(x[:, cols].reshape(NPL, HW)),
                "gg": np.ascontiguousarray(ggf[:, cols].reshape(NT, P).T),
                "bg": np.ascontiguousarray(bgf[:, cols].reshape(NT, P).T),
                "gglh": np.ascontiguousarray(
                    (ALPHA * gamma_global[cols]).reshape(8, 1)),
                "bglh": np.ascontiguousarray(
                    ((1.0 - ALPHA) * beta_global[cols]).reshape(8, 1)),
                "mask8": m8,
                "mask8t": m8t,
            }
        )

    res = run_bass_kernel_spmd(
        nc, in_maps, core_ids=list(range(N_CORES)), trace=_trace
    )
    _STATE["last_res"] = res

    out = np.empty((B, C, H, W), dtype=np.float32)
    for i in range(N_CORES):
        cols = slice(i * C_LOC, (i + 1) * C_LOC)
        out[:, cols] = res.results[i]["out"].reshape(B, C_LOC, H, W)
    return out


# revision 25
# speedup vs baseline: 1.1531x; 1.0065x over previous
"""Trainium2 Bass kernel for nn_CalibratedNormFixedAlpha (moe_routing).

Math (reference):
  out = (1-a)*x_global + a*x_groups,  a = 0.5
  x_global = (x - mu_g[c]) * (gamma_global[c] * rsqrt(var_g[c]+eps)) + beta_global[c]
             with mu_g/var_g per-channel over (N,H,W)  [biased var]
  x_groups = (x - mu_s[b,c]) * rsqrt(var_s[b,c]+eps) * g[b,c] + bt[b,c]
             with mu_s/var_s per-(sample,channel) over (H,W)
             g/bt routed from [G,C] tables by labels[b] % G

Everything is affine in x per (b,c):
  out = A[b,c] * x + B[b,c]
  A = 0.5*sg[c] + 0.5*rs[b,c]*g[b,c]
  B = 0.5*(beta_global[c] - mu_g[c]*sg[c]) + 0.5*(bt[b,c] - mu_s[b,c]*rs[b,c]*g[b,c])
  sg = gamma_global*rsqrt(var_g+eps), rs = rsqrt(var_s+eps)

Sharding: CHANNEL-parallel (8 channels/core, all 64 samples). Both stat
families are then core-local: per-(b,c) spatial stats live on one core, and
the global per-channel (N,H,W) stats only mix the 64 sample-planes of a
channel that all sit on the same core -> ZERO collectives.

Per-core shard: 512 (b,c) planes x 12544 spatial, plane = b*8 + c_local,
resident in SBUF as 4 tiles of [128, 12544] f32. Pass 1: bn_stats/bn_aggr
per partition -> (mean, var) per plane; packsum accumulates (mean, E[x^2])
over tiles. The cross-plane (partition-dim) reduction per channel is one
tiny PE matmul with a 0/1 mask (mask8[p,c] = [p%8==c]); a second tiny
matmul broadcasts the per-channel global coefficients back to all 128
partitions. Pass 2: out = A*x + B per partition, stored from SBUF.
HBM traffic = 1 read + 1 write of x.
"""

import numpy as np

# -------- problem constants (hardcoded per contract) --------
B, C, H, W = 64, 64, 112, 112
HW = H * W                 # 12544
N_CORES = 8
C_LOC = C // N_CORES       # 8 channels per core
P = 128                    # SBUF partitions
NPL = B * C_LOC            # 512 planes per core
NT = NPL // P              # 4 tiles of [128, HW]
CHUNK = 1792               # DMA / affine chunk along free dim (7 per tile)
NCH = HW // CHUNK          # 7
SUB = 448                  # bn_stats subgroup (<=512, uniform size)
NSUB = CHUNK // SUB        # 4
EPS = 1e-5
ALPHA = 0.5
NUM_GROUPS = 32

_STATE = {}


def _build_module(chunk=CHUNK, sub=SUB, load_eng="sync", store_eng="sync",
                  store_chunk=None, first_tile_act=True, first_store_split=4):
    import concourse.bass as bass
    import concourse.bacc as bacc
    import concourse.tile as tile
    from concourse import mybir

    nch = HW // chunk
    nsub = chunk // sub
    if store_chunk is None:
        store_chunk = chunk
    f32 = mybir.dt.float32
    nc = bacc.Bacc(
        "TRN2",
        target_bir_lowering=False,
        debug=False,
        num_devices=N_CORES,
        dynamic_dma_scratch_size=8192,
    )

    x_h = nc.dram_tensor("x", [NPL, HW], f32, kind="ExternalInput")
    gg_h = nc.dram_tensor("gg", [P, NT], f32, kind="ExternalInput")     # routed gamma per plane
    bg_h = nc.dram_tensor("bg", [P, NT], f32, kind="ExternalInput")     # routed beta per plane
    ggl_h = nc.dram_tensor("gglh", [8, 1], f32, kind="ExternalInput")   # 0.5*gamma_global[core ch]
    bgl_h = nc.dram_tensor("bglh", [8, 1], f32, kind="ExternalInput")   # 0.5*beta_global[core ch]
    m8_h = nc.dram_tensor("mask8", [P, 8], f32, kind="ExternalInput")   # [p%8==c]
    m8t_h = nc.dram_tensor("mask8t", [8, P], f32, kind="ExternalInput")  # transpose
    out_h = nc.dram_tensor("out", [NPL, HW], f32, kind="ExternalOutput")

    x_ap = x_h.ap()
    out_ap = out_h.ap()
    Sqrt = mybir.ActivationFunctionType.Sqrt
    Identity = mybir.ActivationFunctionType.Identity
    add = mybir.AluOpType.add
    mult = mybir.AluOpType.mult

    with tile.TileContext(nc) as tc:
        with (
            tc.tile_pool(name="xp", bufs=1) as xp,
            tc.tile_pool(name="sp", bufs=1) as sp,
            tc.tile_pool(name="pp", bufs=1, space="PSUM") as pp,
        ):
            # small replicated inputs
            gg_sb = sp.tile([P, NT], f32, tag="gg")
            bg_sb = sp.tile([P, NT], f32, tag="bg")
            ggl_sb = sp.tile([8, 1], f32, tag="ggl")
            bgl_sb = sp.tile([8, 1], f32, tag="bgl")
            m8_sb = sp.tile([P, 8], f32, tag="m8")
            m8t_sb = sp.tile([8, P], f32, tag="m8t")
            nc.gpsimd.dma_start(out=gg_sb[:, :], in_=gg_h.ap())
            nc.gpsimd.dma_start(out=bg_sb[:, :], in_=bg_h.ap())
            nc.gpsimd.dma_start(out=ggl_sb[:, :], in_=ggl_h.ap())
            nc.gpsimd.dma_start(out=bgl_sb[:, :], in_=bgl_h.ap())
            nc.gpsimd.dma_start(out=m8_sb[:, :], in_=m8_h.ap())
            nc.gpsimd.dma_start(out=m8t_sb[:, :], in_=m8t_h.ap())
            eps_sb = sp.tile([P, 1], f32, tag="eps", name="eps")
            nc.vector.memset(eps_sb[:, :], EPS)

            xt = [xp.tile([P, HW], f32, tag=f"x{t}", name=f"x{t}") for t in range(NT)]
            stats = [sp.tile([P, nch * nsub + 1, 6], f32, tag=f"st{t}", name=f"st{t}")
                     for t in range(NT)]
            mv = [sp.tile([P, 2], f32, tag=f"mv{t}", name=f"mv{t}") for t in range(NT)]
            pre_all = sp.tile([P, NT], f32, tag="pre")   # 0.5*rs*g per tile col
            c2_all = sp.tile([P, NT], f32, tag="c2")     # 0.5*(bt - mean*rs*g)
            pks = [None] * NT

            # ---- pass 1: load + per-plane stats ----
            for t in range(NT):
                rows = slice(t * P, (t + 1) * P)
                si = 0  # subgroup index into stats[t]
                for ch in range(nch):
                    base = ch * chunk
                    if t == NT - 1:
                        # the whole last tile arrives as per-subgroup
                        # mini-loads: bn_stats then starts at arrival+sem for
                        # every subgroup and never lumps a full chunk's worth
                        # of stats behind the final bytes; the very last
                        # subgroups are halved to shorten the critical path
                        widths = ([sub] * nsub if ch < nch - 1
                                  else [sub] * (nsub - 1) + [sub // 2, sub // 2])
                        off = base
                        for w in widths:
                            ssl = slice(off, off + w)
                            off += w
                            le = ["sync", "scalar"][si % 2]
                            getattr(nc, le).dma_start(
                                out=xt[t][:, ssl], in_=x_ap[rows, ssl]
                            )
                            nc.vector.bn_stats(
                                out=stats[t][:, si, :], in_=xt[t][:, ssl]
                            )
                            si += 1
                    else:
                        sl = slice(base, base + chunk)
                        getattr(nc, load_eng).dma_start(out=xt[t][:, sl], in_=x_ap[rows, sl])
                        for s in range(nsub):
                            ssl = slice(base + s * sub, base + (s + 1) * sub)
                            nc.vector.bn_stats(
                                out=stats[t][:, si, :], in_=xt[t][:, ssl]
                            )
                            si += 1
                nc.vector.bn_aggr(out=mv[t][:, :], in_=stats[t][:, 0:si, :])

                # pack (mean, E[x^2]) and accumulate over tiles. Early tiles
                # go through Pool to keep DVE free for the bn_stats stream;
                # the last tile stays on DVE (it is idle post-aggr and the
                # Pool handoff would cost extra sem hops on the critical path)
                eng = nc.vector if t == NT - 1 else nc.gpsimd
                msq = sp.tile([P, 1], f32, tag=f"msq{t}", name=f"msq{t}")
                eng.tensor_mul(out=msq[:, :], in0=mv[t][:, 0:1], in1=mv[t][:, 0:1])
                pk = sp.tile([P, 2], f32, tag=f"pk{t}", name=f"pk{t}")
                eng.tensor_copy(out=pk[:, 0:1], in_=mv[t][:, 0:1])
                eng.tensor_add(out=pk[:, 1:2], in0=mv[t][:, 1:2], in1=msq[:, :])
                if t == 0:
                    pks[0] = pk
                else:
                    acc = sp.tile([P, 2], f32, tag=f"pka{t}", name=f"pka{t}")
                    eng.tensor_add(out=acc[:, :], in0=pks[t - 1][:, :], in1=pk[:, :])
                    pks[t] = acc

                # local coefficient pieces (independent of global stats);
                # gg/bg arrive pre-scaled by ALPHA from the host:
                # pre = 0.5*g*rsqrt(var+eps) = gg/sd, c2 = bg - mean*pre
                sd = sp.tile([P, 1], f32, tag=f"sd{t}", name=f"sd{t}")
                nc.scalar.activation(out=sd[:, :], in_=mv[t][:, 1:2], func=Sqrt, bias=eps_sb[:, 0:1])
                rs = sp.tile([P, 1], f32, tag=f"rs{t}", name=f"rs{t}")
                nc.vector.reciprocal(out=rs[:, :], in_=sd[:, :])
                eng.tensor_mul(out=pre_all[:, t : t + 1], in0=gg_sb[:, t : t + 1], in1=rs[:, :])
                mB = sp.tile([P, 1], f32, tag=f"mB{t}", name=f"mB{t}")
                eng.tensor_mul(out=mB[:, :], in0=mv[t][:, 0:1], in1=pre_all[:, t : t + 1])
                eng.tensor_sub(out=c2_all[:, t : t + 1], in0=bg_sb[:, t : t + 1], in1=mB[:, :])

            # ---- per-channel global stats via mask matmul (PE) ----
            # mask8 is pre-scaled by 1/B host-side, so matmul1 yields
            # (mu_g, E[x^2]_g) per channel directly
            s8_ps = pp.tile([8, 2], f32, tag="s8")
            nc.tensor.matmul(s8_ps[:, :], m8_sb[:, :], pks[NT - 1][:, :])

            # chain on 8 partitions: varg = E[x^2]-mu^2,
            # sgh = 0.5*gg*rsqrt(varg+eps), Kh = 0.5*bg - mu*sgh
            s8 = sp.tile([8, 2], f32, tag="s8sb")
            nc.vector.tensor_copy(out=s8[:, :], in_=s8_ps[:, :])
            musq = sp.tile([8, 1], f32, tag="musq")
            nc.vector.tensor_mul(out=musq[:, :], in0=s8[:, 0:1], in1=s8[:, 0:1])
            varg = sp.tile([8, 1], f32, tag="varg")
            nc.vector.tensor_sub(out=varg[:, :], in0=s8[:, 1:2], in1=musq[:, :])
            sdg = sp.tile([8, 1], f32, tag="sdg")
            nc.scalar.activation(out=sdg[:, :], in_=varg[:, :], func=Sqrt, bias=eps_sb[0:8, 0:1])
            invg = sp.tile([8, 1], f32, tag="invg")
            nc.vector.reciprocal(out=invg[:, :], in_=sdg[:, :])
            gk = sp.tile([8, 2], f32, tag="gk")
            nc.vector.tensor_mul(out=gk[:, 0:1], in0=ggl_sb[:, :], in1=invg[:, :])  # sgh
            kt = sp.tile([8, 1], f32, tag="kt")
            nc.vector.tensor_mul(out=kt[:, :], in0=s8[:, 0:1], in1=gk[:, 0:1])
            nc.vector.tensor_sub(out=gk[:, 1:2], in0=bgl_sb[:, :], in1=kt[:, :])    # Kh

            # broadcast per-channel (sgh, Kh) to all 128 partitions (PE)
            gkb_ps = pp.tile([P, 2], f32, tag="gkb")
            nc.tensor.matmul(gkb_ps[:, :], m8t_sb[:, :], gk[:, :])

            # final per-(plane,tile) affine coefficients, all on DVE so the
            # PSUM copy and both adds run back-to-back with no cross-engine
            # hops (DVE is idle once the stats stream has drained)
            gkb = sp.tile([P, 2], f32, tag="gkbs")
            nc.vector.tensor_copy(out=gkb[:, :], in_=gkb_ps[:, :])
            A_all = sp.tile([P, NT], f32, tag="Aall")
            B_all = sp.tile([P, NT], f32, tag="Ball")
            nc.vector.tensor_scalar_add(out=A_all[:, :], in0=pre_all[:, :], scalar1=gkb[:, 0:1])
            nc.vector.tensor_scalar_add(out=B_all[:, :], in0=c2_all[:, :], scalar1=gkb[:, 1:2])

            # ---- pass 2: out = A*x + B, store ----
            for t in range(NT):
                rows = slice(t * P, (t + 1) * P)
                for ch in range(nch):
                    sl = slice(ch * chunk, (ch + 1) * chunk)
                    first = t == 0 and ch == 0
                    on_act = first_tile_act and t == 0
                    if first and first_store_split > 1:
                        # split the first chunk so the store pipeline launches
                        # as early as possible; the first two sub-affines run
                        # on DVE right behind the A/B computation (same
                        # engine, no sem hop), the rest on ACT
                        fs = chunk // first_store_split
                        for s in range(first_store_split):
                            fsl = slice(s * fs, (s + 1) * fs)
                            if s < 2:
                                nc.vector.tensor_scalar(
                                    out=xt[t][:, fsl], in0=xt[t][:, fsl],
                                    scalar1=A_all[:, t : t + 1],
                                    scalar2=B_all[:, t : t + 1],
                                    op0=mult, op1=add,
                                )
                            else:
                                nc.scalar.activation(
                                    out=xt[t][:, fsl], in_=xt[t][:, fsl], func=Identity,
                                    bias=B_all[:, t : t + 1], scale=A_all[:, t : t + 1],
                                )
                            getattr(nc, ["sync", "scalar"][s % 2]).dma_start(
                                out=out_ap[rows, fsl], in_=xt[t][:, fsl]
                            )
                        continue
                    if on_act:
                        nc.scalar.activation(
                            out=xt[t][:, sl], in_=xt[t][:, sl], func=Identity,
                            bias=B_all[:, t : t + 1], scale=A_all[:, t : t + 1],
                        )
                    else:
                        nc.vector.tensor_scalar(
                            out=xt[t][:, sl], in0=xt[t][:, sl],
                            scalar1=A_all[:, t : t + 1], scalar2=B_all[:, t : t + 1],
                            op0=mult, op1=add,
                        )
                    for sc in range(chunk // store_chunk):
                        ssl = slice(ch * chunk + sc * store_chunk,
                                    ch * chunk + (sc + 1) * store_chunk)
                        getattr(nc, store_eng).dma_start(
                            out=out_ap[rows, ssl], in_=xt[t][:, ssl]
                        )

    nc.compile()
    return nc


def _get_nc():
    if "nc" not in _STATE:
        _STATE["nc"] = _build_module()
    return _STATE["nc"]


def kernel(x, gamma_global, beta_global, gamma_groups, beta_groups, labels,
           _trace=False):
    from concourse.bass_utils import run_bass_kernel_spmd

    nc = _get_nc()

    x = np.asarray(x, dtype=np.float32)
    gamma_global = np.asarray(gamma_global, dtype=np.float32)
    beta_global = np.asarray(beta_global, dtype=np.float32)
    gamma_groups = np.asarray(gamma_groups, dtype=np.float32)
    beta_groups = np.asarray(beta_groups, dtype=np.float32)
    labels = np.asarray(labels)

    # host-side routing (tiny): per-(b,c) affine params
    gidx = (labels.astype(np.int64) % NUM_GROUPS)
    ggf = gamma_groups[gidx]  # [B, C]
    bgf = beta_groups[gidx]   # [B, C]

    pidx = np.arange(P)
    m8full = (pidx[:, None] % 8 == np.arange(8)[None, :]).astype(np.float32)
    m8 = m8full * (1.0 / B)          # folds the /B of the global mean/E[x^2]
    m8t = np.ascontiguousarray(m8full.T)

    in_maps = []
    for i in range(N_CORES):
        cols = slice(i * C_LOC, (i + 1) * C_LOC)
        # plane = b*8 + c_local
        in_maps.append(
            {
                "x": np.ascontiguousarray(x[:, cols].reshape(NPL, HW)),
                "gg": np.ascontiguousarray(ALPHA * ggf[:, cols].reshape(NT, P).T),
                "bg": np.ascontiguousarray(ALPHA * bgf[:, cols].reshape(NT, P).T),
                "gglh": np.ascontiguousarray(
                    (ALPHA * gamma_global[cols]).reshape(8, 1)),
                "bglh": np.ascontiguousarray(
                    ((1.0 - ALPHA) * beta_global[cols]).reshape(8, 1)),
                "mask8": m8,
                "mask8t": m8t,
            }
        )

    res = run_bass_kernel_spmd(
        nc, in_maps, core_ids=list(range(N_CORES)), trace=_trace
    )
    _STATE["last_res"] = res

    out = np.empty((B, C, H, W), dtype=np.float32)
    for i in range(N_CORES):
        cols = slice(i * C_LOC, (i + 1) * C_LOC)
        out[:, cols] = res.results[i]["out"].reshape(B, C_LOC, H, W)
    return out


# revision 32
# speedup vs baseline: 1.1554x; 1.0020x over previous
"""Trainium2 Bass kernel for nn_CalibratedNormFixedAlpha (moe_routing).

Math (reference):
  out = (1-a)*x_global + a*x_groups,  a = 0.5
  x_global = (x - mu_g[c]) * (gamma_global[c] * rsqrt(var_g[c]+eps)) + beta_global[c]
             with mu_g/var_g per-channel over (N,H,W)  [biased var]
  x_groups = (x - mu_s[b,c]) * rsqrt(var_s[b,c]+eps) * g[b,c] + bt[b,c]
             with mu_s/var_s per-(sample,channel) over (H,W)
             g/bt routed from [G,C] tables by labels[b] % G

Everything is affine in x per (b,c):
  out = A[b,c] * x + B[b,c]
  A = 0.5*sg[c] + 0.5*rs[b,c]*g[b,c]
  B = 0.5*(beta_global[c] - mu_g[c]*sg[c]) + 0.5*(bt[b,c] - mu_s[b,c]*rs[b,c]*g[b,c])
  sg = gamma_global*rsqrt(var_g+eps), rs = rsqrt(var_s+eps)

Sharding: CHANNEL-parallel (8 channels/core, all 64 samples). Both stat
families are then core-local: per-(b,c) spatial stats live on one core, and
the global per-channel (N,H,W) stats only mix the 64 sample-planes of a
channel that all sit on the same core -> ZERO collectives.

Per-core shard: 512 (b,c) planes x 12544 spatial, plane = b*8 + c_local,
resident in SBUF as 4 tiles of [128, 12544] f32. Pass 1: bn_stats/bn_aggr
per partition -> (mean, var) per plane; packsum accumulates (mean, E[x^2])
over tiles. The cross-plane (partition-dim) reduction per channel is one
tiny PE matmul with a 0/1 mask (mask8[p,c] = [p%8==c]); a second tiny
matmul broadcasts the per-channel global coefficients back to all 128
partitions. Pass 2: out = A*x + B per partition, stored from SBUF.
HBM traffic = 1 read + 1 write of x.
"""

import numpy as np

# -------- problem constants (hardcoded per contract) --------
B, C, H, W = 64, 64, 112, 112
HW = H * W                 # 12544
N_CORES = 8
C_LOC = C // N_CORES       # 8 channels per core
P = 128                    # SBUF partitions
NPL = B * C_LOC            # 512 planes per core
NT = NPL // P              # 4 tiles of [128, HW]
CHUNK = 1792               # DMA / affine chunk along free dim (7 per tile)
NCH = HW // CHUNK          # 7
SUB = 448                  # bn_stats subgroup (<=512, uniform size)
NSUB = CHUNK // SUB        # 4
EPS = 1e-5
ALPHA = 0.5
NUM_GROUPS = 32

_STATE = {}


def _build_module(chunk=CHUNK, sub=SUB, load_eng="sync", store_eng="sync",
                  store_chunk=None, first_tile_act=True, first_store_split=4):
    import concourse.bass as bass
    import concourse.bacc as bacc
    import concourse.tile as tile
    from concourse import mybir

    nch = HW // chunk
    nsub = chunk // sub
    if store_chunk is None:
        store_chunk = chunk
    f32 = mybir.dt.float32
    nc = bacc.Bacc(
        "TRN2",
        target_bir_lowering=False,
        debug=False,
        num_devices=N_CORES,
        dynamic_dma_scratch_size=8192,
    )

    x_h = nc.dram_tensor("x", [NPL, HW], f32, kind="ExternalInput")
    gg_h = nc.dram_tensor("gg", [P, NT], f32, kind="ExternalInput")     # routed gamma per plane
    bg_h = nc.dram_tensor("bg", [P, NT], f32, kind="ExternalInput")     # routed beta per plane
    ggl_h = nc.dram_tensor("gglh", [8, 1], f32, kind="ExternalInput")   # 0.5*gamma_global[core ch]
    bgl_h = nc.dram_tensor("bglh", [8, 1], f32, kind="ExternalInput")   # 0.5*beta_global[core ch]
    m8_h = nc.dram_tensor("mask8", [P, 8], f32, kind="ExternalInput")   # [p%8==c]
    m8t_h = nc.dram_tensor("mask8t", [8, P], f32, kind="ExternalInput")  # transpose
    out_h = nc.dram_tensor("out", [NPL, HW], f32, kind="ExternalOutput")

    x_ap = x_h.ap()
    out_ap = out_h.ap()
    Sqrt = mybir.ActivationFunctionType.Sqrt
    Identity = mybir.ActivationFunctionType.Identity
    add = mybir.AluOpType.add
    mult = mybir.AluOpType.mult

    with tile.TileContext(nc) as tc:
        with (
            tc.tile_pool(name="xp", bufs=1) as xp,
            tc.tile_pool(name="sp", bufs=1) as sp,
            tc.tile_pool(name="pp", bufs=1, space="PSUM") as pp,
        ):
            # small replicated inputs
            gg_sb = sp.tile([P, NT], f32, tag="gg")
            bg_sb = sp.tile([P, NT], f32, tag="bg")
            ggl_sb = sp.tile([8, 1], f32, tag="ggl")
            bgl_sb = sp.tile([8, 1], f32, tag="bgl")
            m8_sb = sp.tile([P, 8], f32, tag="m8")
            m8t_sb = sp.tile([8, P], f32, tag="m8t")
            nc.gpsimd.dma_start(out=gg_sb[:, :], in_=gg_h.ap())
            nc.gpsimd.dma_start(out=bg_sb[:, :], in_=bg_h.ap())
            nc.gpsimd.dma_start(out=ggl_sb[:, :], in_=ggl_h.ap())
            nc.gpsimd.dma_start(out=bgl_sb[:, :], in_=bgl_h.ap())
            nc.gpsimd.dma_start(out=m8_sb[:, :], in_=m8_h.ap())
            nc.gpsimd.dma_start(out=m8t_sb[:, :], in_=m8t_h.ap())
            eps_sb = sp.tile([P, 1], f32, tag="eps", name="eps")
            nc.vector.memset(eps_sb[:, :], EPS)

            xt = [xp.tile([P, HW], f32, tag=f"x{t}", name=f"x{t}") for t in range(NT)]
            stats = [sp.tile([P, nch * nsub + 2, 6], f32, tag=f"st{t}", name=f"st{t}")
                     for t in range(NT)]
            mv = [sp.tile([P, 2], f32, tag=f"mv{t}", name=f"mv{t}") for t in range(NT)]
            pre_all = sp.tile([P, NT], f32, tag="pre")   # 0.5*rs*g per tile col
            c2_all = sp.tile([P, NT], f32, tag="c2")     # 0.5*(bt - mean*rs*g)
            pks = [None] * NT

            # ---- pass 1: load + per-plane stats ----
            for t in range(NT):
                rows = slice(t * P, (t + 1) * P)
                si = 0  # subgroup index into stats[t]
                for ch in range(nch):
                    base = ch * chunk
                    if t == NT - 1:
                        # the whole last tile arrives as per-subgroup
                        # mini-loads: bn_stats then starts at arrival+sem for
                        # every subgroup and never lumps a full chunk's worth
                        # of stats behind the final bytes; the very last
                        # subgroups are halved to shorten the critical path
                        widths = ([sub] * nsub if ch < nch - 1
                                  else [sub] * (nsub - 1) + [sub // 2, sub // 2])
                        off = base
                        for w in widths:
                            ssl = slice(off, off + w)
                            off += w
                            le = ["sync", "scalar"][si % 2]
                            getattr(nc, le).dma_start(
                                out=xt[t][:, ssl], in_=x_ap[rows, ssl]
                            )
                            nc.vector.bn_stats(
                                out=stats[t][:, si, :], in_=xt[t][:, ssl]
                            )
                            si += 1
                    else:
                        sl = slice(base, base + chunk)
                        getattr(nc, load_eng).dma_start(out=xt[t][:, sl], in_=x_ap[rows, sl])
                        for s in range(nsub):
                            ssl = slice(base + s * sub, base + (s + 1) * sub)
                            nc.vector.bn_stats(
                                out=stats[t][:, si, :], in_=xt[t][:, ssl]
                            )
                            si += 1
                nc.vector.bn_aggr(out=mv[t][:, :], in_=stats[t][:, 0:si, :])

                # pack (mean, E[x^2]) and accumulate over tiles. Early tiles
                # go through Pool to keep DVE free for the bn_stats stream;
                # the last tile stays on DVE (it is idle post-aggr and the
                # Pool handoff would cost extra sem hops on the critical path)
                eng = nc.vector if t == NT - 1 else nc.gpsimd
                msq = sp.tile([P, 1], f32, tag=f"msq{t}", name=f"msq{t}")
                eng.tensor_mul(out=msq[:, :], in0=mv[t][:, 0:1], in1=mv[t][:, 0:1])
                pk = sp.tile([P, 2], f32, tag=f"pk{t}", name=f"pk{t}")
                eng.tensor_copy(out=pk[:, 0:1], in_=mv[t][:, 0:1])
                eng.tensor_add(out=pk[:, 1:2], in0=mv[t][:, 1:2], in1=msq[:, :])
                if t == 0:
                    pks[0] = pk
                elif t < NT - 1:
                    acc = sp.tile([P, 2], f32, tag=f"pka{t}", name=f"pka{t}")
                    eng.tensor_add(out=acc[:, :], in0=pks[t - 1][:, :], in1=pk[:, :])
                    pks[t] = acc
                else:
                    pk_last = pk

                # local coefficient pieces (independent of global stats);
                # gg/bg arrive pre-scaled by ALPHA from the host:
                # pre = 0.5*g*rsqrt(var+eps) = gg/sd, c2 = bg - mean*pre
                sd = sp.tile([P, 1], f32, tag=f"sd{t}", name=f"sd{t}")
                nc.scalar.activation(out=sd[:, :], in_=mv[t][:, 1:2], func=Sqrt, bias=eps_sb[:, 0:1])
                rs = sp.tile([P, 1], f32, tag=f"rs{t}", name=f"rs{t}")
                nc.vector.reciprocal(out=rs[:, :], in_=sd[:, :])
                eng.tensor_mul(out=pre_all[:, t : t + 1], in0=gg_sb[:, t : t + 1], in1=rs[:, :])
                mB = sp.tile([P, 1], f32, tag=f"mB{t}", name=f"mB{t}")
                eng.tensor_mul(out=mB[:, :], in0=mv[t][:, 0:1], in1=pre_all[:, t : t + 1])
                eng.tensor_sub(out=c2_all[:, t : t + 1], in0=bg_sb[:, t : t + 1], in1=mB[:, :])

            # ---- per-channel global stats via mask matmul (PE) ----
            # mask8 is pre-scaled by 1/B host-side, so the accumulated matmul
            # yields (mu_g, E[x^2]_g) per channel directly. Tiles 0-2 are
            # reduced while tile 3 is still loading; only the second matmul
            # (tile 3's pack) sits on the critical path.
            s8_ps = pp.tile([8, 2], f32, tag="s8")
            nc.tensor.matmul(s8_ps[:, :], m8_sb[:, :], pks[NT - 2][:, :],
                             start=True, stop=False)
            nc.tensor.matmul(s8_ps[:, :], m8_sb[:, :], pk_last[:, :],
                             start=False, stop=True)

            # chain on 8 partitions: varg = E[x^2]-mu^2,
            # sgh = 0.5*gg*rsqrt(varg+eps), Kh = 0.5*bg - mu*sgh
            s8 = sp.tile([8, 2], f32, tag="s8sb")
            nc.vector.tensor_copy(out=s8[:, :], in_=s8_ps[:, :])
            musq = sp.tile([8, 1], f32, tag="musq")
            nc.vector.tensor_mul(out=musq[:, :], in0=s8[:, 0:1], in1=s8[:, 0:1])
            varg = sp.tile([8, 1], f32, tag="varg")
            nc.vector.tensor_sub(out=varg[:, :], in0=s8[:, 1:2], in1=musq[:, :])
            sdg = sp.tile([8, 1], f32, tag="sdg")
            nc.scalar.activation(out=sdg[:, :], in_=varg[:, :], func=Sqrt, bias=eps_sb[0:8, 0:1])
            invg = sp.tile([8, 1], f32, tag="invg")
            nc.vector.reciprocal(out=invg[:, :], in_=sdg[:, :])
            gk = sp.tile([8, 2], f32, tag="gk")
            nc.vector.tensor_mul(out=gk[:, 0:1], in0=ggl_sb[:, :], in1=invg[:, :])  # sgh
            kt = sp.tile([8, 1], f32, tag="kt")
            nc.vector.tensor_mul(out=kt[:, :], in0=s8[:, 0:1], in1=gk[:, 0:1])
            nc.vector.tensor_sub(out=gk[:, 1:2], in0=bgl_sb[:, :], in1=kt[:, :])    # Kh

            # broadcast per-channel (sgh, Kh) to all 128 partitions (PE)
            gkb_ps = pp.tile([P, 2], f32, tag="gkb")
            nc.tensor.matmul(gkb_ps[:, :], m8t_sb[:, :], gk[:, :])

            # final per-(plane,tile) affine coefficients, all on DVE so the
            # PSUM copy and both adds run back-to-back with no cross-engine
            # hops (DVE is idle once the stats stream has drained)
            gkb = sp.tile([P, 2], f32, tag="gkbs")
            nc.vector.tensor_copy(out=gkb[:, :], in_=gkb_ps[:, :])
            A_all = sp.tile([P, NT], f32, tag="Aall")
            B_all = sp.tile([P, NT], f32, tag="Ball")
            nc.vector.tensor_scalar_add(out=A_all[:, :], in0=pre_all[:, :], scalar1=gkb[:, 0:1])
            nc.vector.tensor_scalar_add(out=B_all[:, :], in0=c2_all[:, :], scalar1=gkb[:, 1:2])

            # ---- pass 2: out = A*x + B, store ----
            for t in range(NT):
                rows = slice(t * P, (t + 1) * P)
                for ch in range(nch):
                    sl = slice(ch * chunk, (ch + 1) * chunk)
                    first = t == 0 and ch == 0
                    on_act = first_tile_act and t == 0
                    if first and first_store_split > 1:
                        # split the first chunk so the store pipeline launches
                        # as early as possible; the first two sub-affines run
                        # on DVE right behind the A/B computation (same
                        # engine, no sem hop), the rest on ACT
                        fs = chunk // first_store_split
                        for s in range(first_store_split):
                            fsl = slice(s * fs, (s + 1) * fs)
                            if s < 2:
                                nc.vector.tensor_scalar(
                                    out=xt[t][:, fsl], in0=xt[t][:, fsl],
                                    scalar1=A_all[:, t : t + 1],
                                    scalar2=B_all[:, t : t + 1],
                                    op0=mult, op1=add,
                                )
                            else:
                                nc.scalar.activation(
                                    out=xt[t][:, fsl], in_=xt[t][:, fsl], func=Identity,
                                    bias=B_all[:, t : t + 1], scale=A_all[:, t : t + 1],
                                )
                            getattr(nc, ["sync", "scalar"][s % 2]).dma_start(
                                out=out_ap[rows, fsl], in_=xt[t][:, fsl]
                            )
                        continue
                    if on_act:
                        nc.scalar.activation(
                            out=xt[t][:, sl], in_=xt[t][:, sl], func=Identity,
                            bias=B_all[:, t : t + 1], scale=A_all[:, t : t + 1],
                        )
                    else:
                        nc.vector.tensor_scalar(
                            out=xt[t][:, sl], in0=xt[t][:, sl],
                            scalar1=A_all[:, t : t + 1], scalar2=B_all[:, t : t + 1],
                            op0=mult, op1=add,
                        )
                    for sc in range(chunk // store_chunk):
                        ssl = slice(ch * chunk + sc * store_chunk,
                                    ch * chunk + (sc + 1) * store_chunk)
                        getattr(nc, store_eng).dma_start(
                            out=out_ap[rows, ssl], in_=xt[t][:, ssl]
                        )

    nc.compile()
    return nc


def _get_nc():
    if "nc" not in _STATE:
        _STATE["nc"] = _build_module()
    return _STATE["nc"]


def kernel(x, gamma_global, beta_global, gamma_groups, beta_groups, labels,
           _trace=False):
    from concourse.bass_utils import run_bass_kernel_spmd

    nc = _get_nc()

    x = np.asarray(x, dtype=np.float32)
    gamma_global = np.asarray(gamma_global, dtype=np.float32)
    beta_global = np.asarray(beta_global, dtype=np.float32)
    gamma_groups = np.asarray(gamma_groups, dtype=np.float32)
    beta_groups = np.asarray(beta_groups, dtype=np.float32)
    labels = np.asarray(labels)

    # host-side routing (tiny): per-(b,c) affine params
    gidx = (labels.astype(np.int64) % NUM_GROUPS)
    ggf = gamma_groups[gidx]  # [B, C]
    bgf = beta_groups[gidx]   # [B, C]

    pidx = np.arange(P)
    m8full = (pidx[:, None] % 8 == np.arange(8)[None, :]).astype(np.float32)
    m8 = m8full * (1.0 / B)          # folds the /B of the global mean/E[x^2]
    m8t = np.ascontiguousarray(m8full.T)

    in_maps = []
    for i in range(N_CORES):
        cols = slice(i * C_LOC, (i + 1) * C_LOC)
        # plane = b*8 + c_local
        in_maps.append(
            {
                "x": np.ascontiguousarray(x[:, cols].reshape(NPL, HW)),
                "gg": np.ascontiguousarray(ALPHA * ggf[:, cols].reshape(NT, P).T),
                "bg": np.ascontiguousarray(ALPHA * bgf[:, cols].reshape(NT, P).T),
                "gglh": np.ascontiguousarray(
                    (ALPHA * gamma_global[cols]).reshape(8, 1)),
                "bglh": np.ascontiguousarray(
                    ((1.0 - ALPHA) * beta_global[cols]).reshape(8, 1)),
                "mask8": m8,
                "mask8t": m8t,
            }
        )

    res = run_bass_kernel_spmd(
        nc, in_maps, core_ids=list(range(N_CORES)), trace=_trace
    )
    _STATE["last_res"] = res

    out = np.empty((B, C, H, W), dtype=np.float32)
    for i in range(N_CORES):
        cols = slice(i * C_LOC, (i + 1) * C_LOC)
        out[:, cols] = res.results[i]["out"].reshape(B, C_LOC, H, W)
    return out
